# revision 76
# baseline (speedup 1.0000x reference)
"""Trainium2 Bass kernel for the CNN-MAD per-class DTW transport cost.

Math (reference):
  mat_cost[n, j] = C1[n] + C2[c_n, j] - 2*C3[n, j],  c_n = classes[n]
    C1[n]    = sum_t rowsum[c_n, t] * r[n,t],   r[n,t] = sum_d X[n,t,d]^2
    C2[c, j] = sum_p colsum[c, p]  * q[j,p],    q[j,p] = sum_d Y[j,p,d]^2
    C3[n, j] = sum_{p,d} XW[n,p,d] * Y[j,p,d],  XW = pi_c.T @ X (warp)

Sharding: 4x2 grid. Core (rr, cj) owns the samples of classes {2rr, 2rr+1}
and the j-half [512*cj, 512*(cj+1)).  The larger class of each pair goes
to slot A, the smaller to slot B; the SPMD-shared caps (capA, capB) are
the max slot sizes over cores (capB unrounded beyond 4), so NL=capA+capB
carries minimal padding.  One SPMD program for all 8 cores; per-core
class structure enters only through data.  The two big contractions (the
DTW warp and the X~Y inner-product field) run on the PE at fp8 DoubleRow
rate; the tiny bias terms C1/C2 (rank-1 row/col corrections) are
host-precomputed and enter each output psum through one rank-3 fp16
augmentation matmul per j-block:
  - warp XW = piS.T @ X, psum evac'd as a pure contiguous copy
    ((d,n)-major layout, -2 prefolded into the shipped Y).
  - C3 flipped to [j-partition, n-free] psum orientation: 4 j-blocks of
    128, 8 DR passes each over k=(p,d); cost scales with n=NL not NY.
  - outputs leave via SWDGE prepare/trigger writebacks (one queue per
    j-block): descriptors are generated early on Pool, each trigger
    fires right after its block's evac, so the post-compute tail is
    trigger+transfer+sem instead of a full HWDGE dispatch chain.
  - a train of cheap dummy matmuls pins pe_busy_start early so the 3us
    PE p-state ramp elapses before the real matmuls start.
"""

import sys

sys.path.insert(0, "/opt/trn_rl_repo")

import numpy as np

N, NY, T, TP, D, C = 1024, 1024, 256, 256, 8, 8
NCORES = 8
NYL = 512  # j columns per core

_cache = {}

# Engine per warp-psum evac, by emission index (a=ACT, d=DVE).
# Pool/GPSIMD cannot read PSUM on real hardware, so only ACT and DVE may
# evacuate psum tiles; Pool carries the writeback preps and triggers.
XW_EVAC = ["d", "a", "d", "a", "d", "a", "d", "a", "d", "a", "d"]
OUT_EVAC = ("a", "d", "a", "d")
N_PRIME = 52  # PE p-state priming matmuls (0 = off)
WB_JBS = (0, 1, 2, 3)  # j-blocks whose output goes via prepare/trigger writeback


def _copy(nc, eng, dst, src):
    if eng == "a":
        return nc.scalar.mul(dst, src, 1.0)
    elif eng == "d":
        return nc.vector.tensor_copy(dst, src)
    else:
        return nc.gpsimd.tensor_copy(dst, src)


def _build(capA, capB):
    import bass_rust as _br
    import concourse.bacc as bacc
    import concourse.mybir as mybir
    import concourse.tile as tile

    f8 = mybir.dt.float8e4
    bf = mybir.dt.bfloat16
    f16 = mybir.dt.float16
    f32 = mybir.dt.float32
    i32 = mybir.dt.int32
    DR = mybir.MatmulPerfMode.DoubleRow
    NL = capA + capB

    nwb = len(WB_JBS)
    nc = bacc.Bacc(
        "TRN2",
        target_bir_lowering=False,
        debug=False,
        num_devices=NCORES,
        num_swdge_queues=max(1, nwb),
    )

    # pxt = piS | X in (d, tc, n) layout: one contiguous DMA covers piS and
    # the first-half (d<4) warp operand, so the warp starts one transfer in.
    PXT = 1024 + 16 * NL
    pxt_d = nc.dram_tensor("pxt", [128, PXT], f8, kind="ExternalInput")
    ytl_d = nc.dram_tensor("ytl", [128, 16 * NYL], f8, kind="ExternalInput")
    aux_d = nc.dram_tensor("aux", [4, NYL + NL + 16], f16, kind="ExternalInput")
    out_d = nc.dram_tensor("out", [NYL, NL], bf, kind="ExternalOutput")

    with tile.TileContext(nc) as tc:
        with (
            tc.tile_pool(name="io", bufs=1) as pio,
            tc.tile_pool(name="work", bufs=1) as pw,
            tc.tile_pool(name="small", bufs=1) as psm,
            tc.tile_pool(name="ps", bufs=1, space="PSUM") as pp,
        ):
            pxt = pio.tile([128, PXT], f8, tag="pxt")
            ytl = pio.tile([128, 16 * NYL], f8, tag="ytl")
            aux = psm.tile([4, NYL + NL + 16], f16, tag="aux")
            outsb = pw.tile([128, 4 * NL], bf, tag="outsb")

            piSv = pxt[:, 0:1024].rearrange("l (c t p) -> l c t p", c=2, t=2)
            xt2v = pxt[:, 1024:PXT].rearrange("l (d t n) -> l d t n", d=8, t=2)
            ytlv = ytl.rearrange("l (jb kc j) -> l jb kc j", jb=4, kc=16)

            augL = aux[0:3, 0:NYL]            # [c2A | c2B | ones] over j
            augR = aux[0:3, NYL : NYL + NL]   # [indA | indB | c1c] over n

            # ---- writeback preps (descriptor gen; data read at trigger) ---
            wb_sems, wb_prep, wb_trg = {}, {}, {}
            if nwb:
                idxs = psm.tile([128, 2], i32, tag="wbidx")
                nc.gpsimd.memset(idxs[:], 0)
                outv = outsb.rearrange("j (jb o b n) -> j jb o b n", jb=4, o=1, b=2)
                odv = out_d.rearrange("(jb j o) (b n) -> jb b j o n", jb=4, o=1, b=2)
                for jb in sorted(WB_JBS):
                    qn = sorted(WB_JBS).index(jb)
                    sem = nc.alloc_semaphore(f"wbdma{jb}")
                    wb_sems[jb] = sem
                    wb_prep[jb] = nc.gpsimd.kv_writeback(
                        odv[jb],
                        outv[:, jb],
                        idxs[:],
                        prepare_only=True,
                        sem=sem,
                        queue_num=qn,
                    ).ins

            # ---- input DMAs (all SP HWDGE, wire order = emission order) ---
            HPX = 1024 + 8 * NL  # piS + d<4 half of X
            if getattr(sys.modules[__name__], "PXT_SPLIT3", False):
                HP1 = 1024 + 4 * NL  # piS + d0,d1
                nc.sync.dma_start(pxt[:, 0:HP1], pxt_d[:, 0:HP1])
                nc.sync.dma_start(pxt[:, HP1:HPX], pxt_d[:, HP1:HPX])
            else:
                nc.sync.dma_start(pxt[:, 0:HPX], pxt_d[:, 0:HPX])
            nc.sync.dma_start(pxt[:, HPX:PXT], pxt_d[:, HPX:PXT])
            nc.sync.dma_start(aux[:], aux_d[:, :])
            ytldv = ytl_d.rearrange("l (jb x) -> l jb x", jb=4)
            ytlsv = ytl.rearrange("l (jb x) -> l jb x", jb=4)
            for jb in range(4):
                nc.sync.dma_start(ytlsv[:, jb], ytldv[:, jb])

            # ---- PE p-state priming (dummy matmuls on scratch) ------------
            # pe_busy_start is pinned by the FIRST matmul and survives sub-us
            # idle gaps; a train of cheap dummies bridges until real work so
            # the 3us ramp elapses before the warp starts.
            if N_PRIME:
                dum = psm.tile([128, 256], f8, tag="dum")
                nc.vector.memset(dum[:], 1.0)
                dumv = dum.rearrange("l (t o) -> l t o", o=128)
                # all dummies hit ONE psum tile: the WAW chain is free on the
                # in-order PE, while buffer rotation would add sem waits.
                dps = pp.tile([1, 128], f32, tag="psO", bufs=3, name="dps")
                for i in range(N_PRIME):
                    nc.tensor.matmul(
                        dps[:], dumv[:, :, 0:1], dumv,
                        start=True, stop=True, perf_mode=DR,
                        skip_group_check=True,
                    )

            # ---- aug matmuls: psum group starters -------------------------
            # psO bufs=3 holds jb0-2; outps3 is allocated from the psW pool
            # (by the time its WAR resolves the warp psum cycle is drained).
            outps = [
                pp.tile([128, NL], f32, tag="psO", bufs=3, name=f"outps{jb}")
                for jb in range(3)
            ]

            def aug(jb):
                nc.tensor.matmul(
                    outps[jb][:],
                    augL[:, jb * 128 : (jb + 1) * 128],
                    augR,
                    start=True, stop=False, skip_group_check=True,
                )

            # ---- warp (PE) + contiguous evacs -----------------------------
            # 32 slot-matmuls of 144 cols, grouped 3-per-psum-tile (432 cols)
            # so each evac is one contiguous [128, 432] copy into xwt.
            # Group g covers units [3g, 3g+3), unit u = kc*2 + s.
            # Emission order: groups that only need the first pxt DMA first.
            xwt = pw.tile([128, 16 * NL], f8, tag="xwt")
            units = [(u // 2 // 8, (u // 2) % 8, u % 2) for u in range(32)]
            if getattr(sys.modules[__name__], "G2", False):
                gorder = [0, 1, 6, 7, 2, 3, 8, 9, 4, 5, 10]
            else:
                gorder = [0, 1, 6, 7, 2, 3, 4, 5, 8, 9, 10]

            # warp psum tiles cycle through BOTH pools (the jb psums are idle
            # until the augs, which anyway wait for the warp to drain) -- 8
            # effective slots instead of 5 keeps the WAR loop off the path.
            WTAG = ["psW", "psW", "psW", "psW", "psW", "psO", "psO", "psO",
                    "psW", "psW", "psW"]

            caps = (capA, capB)
            uoff = [0]
            for u in range(32):
                uoff.append(uoff[-1] + caps[u % 2])

            def warp(ei, g):
                u0 = 3 * g
                grp = units[u0 : u0 + 3]
                gw = uoff[u0 + len(grp)] - uoff[u0]
                w = pp.tile(
                    [128, gw], f32, tag=WTAG[ei],
                    bufs=5 if WTAG[ei] == "psW" else 3, name=f"xw{g}"
                )
                c0 = 0
                for i, (pc, d, s) in enumerate(grp):
                    so = s * capA
                    nc.tensor.matmul(
                        w[:, c0 : c0 + caps[s]],
                        piSv[:, s, :, pc * 128 : (pc + 1) * 128],
                        xt2v[:, d, :, so : so + caps[s]],
                        start=True, stop=True, perf_mode=DR,
                        skip_group_check=True,
                    )
                    c0 += caps[s]
                _copy(nc, XW_EVAC[ei],
                      xwt[:, uoff[u0] : uoff[u0] + gw], w[:])

            for ei, g in enumerate(gorder):
                warp(ei, g)
            outps.append(pp.tile([128, NL], f32, tag="psW", bufs=5, name="outps3"))
            for jb in range(4):
                aug(jb)
            xwtv = xwt.rearrange("l (kc n) -> l kc n", kc=16)

            # ---- C3: 8 DR passes per j-block, ordered by when the warp
            # groups covering each kc-pair land (see gorder). Rounds are
            # interleaved across j-blocks so only the last two rounds sit
            # behind the final warp evacs; jb3's early rounds slot in after
            # its (last) ytl chunk arrives.
            if getattr(sys.modules[__name__], "G2", False):
                korder = [0, 5, 1, 2, 6, 3, 4, 7]
            else:
                korder = [0, 5, 1, 2, 3, 4, 6, 7]

            def c3_pass(jb, ki):
                k = korder[ki]
                nc.tensor.matmul(
                    outps[jb][:],
                    ytlv[:, jb, 2 * k : 2 * k + 2, :],
                    xwtv[:, 2 * k : 2 * k + 2, :],
                    start=False, stop=(ki == 7), perf_mode=DR,
                    skip_group_check=True,
                )

            def c3_finish(jb):
                if jb == 3 and getattr(sys.modules[__name__], "JB3_SPLIT", False):
                    evs = [
                        _copy(nc, "a", outsb[:, jb * NL : jb * NL + capA],
                              outps[jb][:, 0:capA]),
                        _copy(nc, "d", outsb[:, jb * NL + capA : (jb + 1) * NL],
                              outps[jb][:, capA:NL]),
                    ]
                else:
                    evs = [_copy(nc, OUT_EVAC[jb],
                                 outsb[:, jb * NL : (jb + 1) * NL], outps[jb][:])]
                if jb in wb_sems:
                    # Drop the bogus WAR edge evac->prep (Tile attributes the
                    # prep's deferred outsb read to DMA completion, which
                    # would deadlock against the evac that PRODUCES the data).
                    # Real ordering: evac -> (sync dep) -> trigger -> DMA read.
                    qn = sorted(WB_JBS).index(jb)
                    dep = _br.InstructionNameOrderedSet()
                    for ev in evs:
                        ev.ins.remove_dependency(wb_prep[jb].name)
                        dep.add(ev.ins.name)
                    trg = nc.gpsimd.trigger_dma(count=None, queue_num=qn)
                    trg.ins.add_sync_dependencies_from(dep)
                    wb_trg[jb] = trg.ins
                else:
                    nc.sync.dma_start(
                        out_d[jb * 128 : (jb + 1) * 128, :],
                        outsb[:, jb * NL : (jb + 1) * NL],
                    )

            order = getattr(sys.modules[__name__], "C3_ORDER", 0)
            if order == 0:
                for ki in range(8):
                    for jb in range(3):
                        c3_pass(jb, ki)
                for jb in range(3):
                    c3_finish(jb)
                for ki in range(8):
                    c3_pass(3, ki)
                c3_finish(3)
            elif order == 1:
                # jb3's early rounds slot between jb0-2's finishes
                for ki in range(8):
                    for jb in range(3):
                        c3_pass(jb, ki)
                for ki in range(6):
                    c3_pass(3, ki)
                for jb in range(3):
                    c3_finish(jb)
                for ki in (6, 7):
                    c3_pass(3, ki)
                c3_finish(3)
            else:
                # jb0 drains fully first so its evac/trigger leave earliest
                for ki in range(8):
                    c3_pass(0, ki)
                c3_finish(0)
                for ki in range(8):
                    for jb in (1, 2):
                        c3_pass(jb, ki)
                for jb in (1, 2):
                    c3_finish(jb)
                for ki in range(8):
                    c3_pass(3, ki)
                c3_finish(3)

            # end-of-kernel: hold Pool until every writeback DMA completed
            # (replaces the DMASW lane waits stripped below, which the
            # timeline scheduler cannot satisfy for user-sem'd preps).
            # All completion waits anchor AFTER the LAST trigger: a wait
            # placed between triggers would serialize them by ~1us each.
            last_jb = max(WB_JBS) if nwb else None
            wge_names = {}
            for jb in sorted(WB_JBS):
                wge = nc.gpsimd.wait_ge(wb_sems[jb], 16)
                wge_names[jb] = wge.ins.name
                dep = _br.InstructionNameOrderedSet()
                dep.add(wb_trg[jb].name)
                dep.add(wb_trg[last_jb].name)
                wge.ins.add_sync_dependencies_from(dep)
            for jb in sorted(WB_JBS):
                desc = _br.InstructionNameOrderedSet()
                desc.add(wge_names[jb])
                if jb == last_jb:
                    for j2 in sorted(WB_JBS):
                        desc.add(wge_names[j2])
                wb_trg[jb].descendants = desc

    if nwb:
        for b in nc.m.functions[0].blocks:
            for i in b.instructions:
                si = i.sync_info
                if si is None:
                    continue
                ws = list(si.on_wait)
                if any("DMASW" in str(w) for w in ws):
                    si.on_wait = [w for w in ws if "DMASW" not in str(w)]

    nc.compile()
    return nc


def kernel(X, Y, pi_dtw, classes):
    import ml_dtypes
    from concourse.bass_utils import run_bass_kernel_spmd

    f8 = ml_dtypes.float8_e4m3
    X = np.ascontiguousarray(np.asarray(X, dtype=np.float32))
    Y = np.ascontiguousarray(np.asarray(Y, dtype=np.float32))
    pi_dtw = np.ascontiguousarray(np.asarray(pi_dtw, dtype=np.float32))
    classes = np.asarray(classes).astype(np.int64)

    counts = np.bincount(classes, minlength=C)
    # slot A holds the larger class of each pair, slot B the smaller, so the
    # SPMD-shared caps (and with them every evac/pass column count) shrink.
    pairs = [(2 * r, 2 * r + 1) for r in range(4)]
    slots = [(a, b) if counts[a] >= counts[b] else (b, a) for a, b in pairs]
    rup = lambda v, m: int(-(-int(v) // m) * m)
    capA = rup(max(counts[a] for a, b in slots), 16)
    capB = rup(max(counts[b] for a, b in slots), 4)
    NL = capA + capB

    if (capA, capB) not in _cache:
        _cache[(capA, capB)] = _build(capA, capB)
    nc = _cache[(capA, capB)]

    idx = [np.nonzero(classes == c)[0] for c in range(C)]

    # bias terms (host): row/col norms contracted with the pi sums
    qfull = (Y * Y).sum(axis=2)          # [NY, TP]
    rfull = (X * X).sum(axis=2)          # [N, T]
    colsum = pi_dtw.sum(axis=1)          # [C, TP]
    rowsum = pi_dtw.sum(axis=2)          # [C, T]
    C2 = qfull @ colsum.T                # [NY, C]
    C1 = (rfull * rowsum[classes]).sum(axis=1)  # [N]

    # per j-half: ytl (-2Y, [p_in, jb, pc, d, jj])
    ytls = []
    for cj in range(2):
        Yh = -2.0 * Y[cj * NYL : (cj + 1) * NYL]
        B = Yh.reshape(4, 128, 2, 128, D).transpose(3, 0, 2, 4, 1)
        ytls.append(np.ascontiguousarray(B.reshape(128, 16 * NYL)).astype(f8))

    in_maps = []
    for r in range(4):
        ca, cb = slots[r]
        Xp = np.zeros((NL, T, D), dtype=np.float32)
        Xp[0 : counts[ca]] = X[idx[ca]]
        Xp[capA : capA + counts[cb]] = X[idx[cb]]
        # [t_in, d, tc, n]
        xt2 = Xp.reshape(NL, 2, 128, D).transpose(2, 3, 1, 0).reshape(128, 16 * NL)

        P = pi_dtw[[ca, cb]]
        pis = P.reshape(2, 2, 128, 256).transpose(2, 0, 1, 3).reshape(128, 1024)
        pxt = np.ascontiguousarray(
            np.concatenate([pis, xt2], axis=1)
        ).astype(f8)

        c1c = np.zeros(NL, dtype=np.float32)
        c1c[0 : counts[ca]] = C1[idx[ca]]
        c1c[capA : capA + counts[cb]] = C1[idx[cb]]

        for cj in range(2):
            aux = np.zeros((4, NYL + NL + 16), dtype=np.float16)
            aux[0, 0:NYL] = C2[cj * NYL : (cj + 1) * NYL, ca]
            aux[1, 0:NYL] = C2[cj * NYL : (cj + 1) * NYL, cb]
            aux[2, 0:NYL] = 1.0
            aux[0, NYL : NYL + counts[ca]] = 1.0  # indA
            aux[1, NYL + capA : NYL + capA + counts[cb]] = 1.0  # indB
            aux[2, NYL : NYL + NL] = c1c
            in_maps.append(
                {"pxt": pxt, "ytl": ytls[cj], "aux": aux}
            )

    res = run_bass_kernel_spmd(nc, in_maps, core_ids=list(range(NCORES)))

    out = np.empty((N, NY), dtype=np.float32)
    jr = [np.arange(0, NYL), np.arange(NYL, NY)]
    for r in range(4):
        ca, cb = slots[r]
        for cj in range(2):
            blk = np.asarray(res.results[2 * r + cj]["out"]).astype(np.float32)
            out[np.ix_(idx[ca], jr[cj])] = blk[:, 0 : counts[ca]].T
            out[np.ix_(idx[cb], jr[cj])] = blk[:, capA : capA + counts[cb]].T
    return out


# revision 79
# speedup vs baseline: 1.0151x; 1.0151x over previous
"""Trainium2 Bass kernel for the CNN-MAD per-class DTW transport cost.

Math (reference):
  mat_cost[n, j] = C1[n] + C2[c_n, j] - 2*C3[n, j],  c_n = classes[n]
    C1[n]    = sum_t rowsum[c_n, t] * r[n,t],   r[n,t] = sum_d X[n,t,d]^2
    C2[c, j] = sum_p colsum[c, p]  * q[j,p],    q[j,p] = sum_d Y[j,p,d]^2
    C3[n, j] = sum_{p,d} XW[n,p,d] * Y[j,p,d],  XW = pi_c.T @ X (warp)

Sharding: 4x2 grid. Core (rr, cj) owns the samples of classes {2rr, 2rr+1}
and the j-half [512*cj, 512*(cj+1)).  The larger class of each pair goes
to slot A, the smaller to slot B; the SPMD-shared caps (capA, capB) are
the max slot sizes over cores (capB unrounded beyond 4), so NL=capA+capB
carries minimal padding.  One SPMD program for all 8 cores; per-core
class structure enters only through data.  The two big contractions (the
DTW warp and the X~Y inner-product field) run on the PE at fp8 DoubleRow
rate; the tiny bias terms C1/C2 (rank-1 row/col corrections) are
host-precomputed and enter each output psum through one rank-3 fp16
augmentation matmul per j-block:
  - warp XW = piS.T @ X, psum evac'd as a pure contiguous copy
    ((d,n)-major layout, -2 prefolded into the shipped Y).
  - C3 flipped to [j-partition, n-free] psum orientation: 4 j-blocks of
    128, 8 DR passes each over k=(p,d); cost scales with n=NL not NY.
  - outputs leave via SWDGE prepare/trigger writebacks (one queue per
    j-block): descriptors are generated early on Pool, each trigger
    fires right after its block's evac, so the post-compute tail is
    trigger+transfer+sem instead of a full HWDGE dispatch chain.
  - a train of cheap dummy matmuls pins pe_busy_start early so the 3us
    PE p-state ramp elapses before the real matmuls start.
"""

import sys

sys.path.insert(0, "/opt/trn_rl_repo")

import numpy as np

N, NY, T, TP, D, C = 1024, 1024, 256, 256, 8, 8
NCORES = 8
NYL = 512  # j columns per core

_cache = {}

# Engine per warp-psum evac, by emission index (a=ACT, d=DVE).
# Pool/GPSIMD cannot read PSUM on real hardware, so only ACT and DVE may
# evacuate psum tiles; Pool carries the writeback preps and triggers.
XW_EVAC = ["d", "a", "d", "a", "d", "a", "a", "a", "d", "a", "d"]
OUT_EVAC = ("a", "d", "a", "d")
N_PRIME = 52  # PE p-state priming matmuls (0 = off)
WB_JBS = (0, 1, 2, 3)  # j-blocks whose output goes via prepare/trigger writeback


def _copy(nc, eng, dst, src):
    if eng == "a":
        return nc.scalar.mul(dst, src, 1.0)
    elif eng == "d":
        return nc.vector.tensor_copy(dst, src)
    else:
        return nc.gpsimd.tensor_copy(dst, src)


def _build(capA, capB):
    import bass_rust as _br
    import concourse.bacc as bacc
    import concourse.mybir as mybir
    import concourse.tile as tile

    f8 = mybir.dt.float8e4
    bf = mybir.dt.bfloat16
    f16 = mybir.dt.float16
    f32 = mybir.dt.float32
    i32 = mybir.dt.int32
    DR = mybir.MatmulPerfMode.DoubleRow
    NL = capA + capB

    nwb = len(WB_JBS)
    nc = bacc.Bacc(
        "TRN2",
        target_bir_lowering=False,
        debug=False,
        num_devices=NCORES,
        num_swdge_queues=max(1, nwb),
    )

    # pxt = piS | X in (d, tc, n) layout: one contiguous DMA covers piS and
    # the first-half (d<4) warp operand, so the warp starts one transfer in.
    PXT = 1024 + 16 * NL
    pxt_d = nc.dram_tensor("pxt", [128, PXT], f8, kind="ExternalInput")
    ytl_d = nc.dram_tensor("ytl", [128, 16 * NYL], f8, kind="ExternalInput")
    aux_d = nc.dram_tensor("aux", [4, NYL + NL + 16], f16, kind="ExternalInput")
    out_d = nc.dram_tensor("out", [NYL, NL], bf, kind="ExternalOutput")

    with tile.TileContext(nc) as tc:
        with (
            tc.tile_pool(name="io", bufs=1) as pio,
            tc.tile_pool(name="work", bufs=1) as pw,
            tc.tile_pool(name="small", bufs=1) as psm,
            tc.tile_pool(name="ps", bufs=1, space="PSUM") as pp,
        ):
            pxt = pio.tile([128, PXT], f8, tag="pxt")
            ytl = pio.tile([128, 16 * NYL], f8, tag="ytl")
            aux = psm.tile([4, NYL + NL + 16], f16, tag="aux")
            outsb = pw.tile([128, 4 * NL], bf, tag="outsb")

            piSv = pxt[:, 0:1024].rearrange("l (c t p) -> l c t p", c=2, t=2)
            xt2v = pxt[:, 1024:PXT].rearrange("l (d t n) -> l d t n", d=8, t=2)
            ytlv = ytl.rearrange("l (jb kc j) -> l jb kc j", jb=4, kc=16)

            augL = aux[0:3, 0:NYL]            # [c2A | c2B | ones] over j
            augR = aux[0:3, NYL : NYL + NL]   # [indA | indB | c1c] over n

            # ---- writeback preps (descriptor gen; data read at trigger) ---
            wb_sems, wb_prep, wb_trg = {}, {}, {}
            if nwb:
                idxs = psm.tile([128, 2], i32, tag="wbidx")
                nc.gpsimd.memset(idxs[:], 0)
                outv = outsb.rearrange("j (jb o b n) -> j jb o b n", jb=4, o=1, b=2)
                odv = out_d.rearrange("(jb j o) (b n) -> jb b j o n", jb=4, o=1, b=2)
                for jb in sorted(WB_JBS):
                    qn = sorted(WB_JBS).index(jb)
                    sem = nc.alloc_semaphore(f"wbdma{jb}")
                    wb_sems[jb] = sem
                    wb_prep[jb] = nc.gpsimd.kv_writeback(
                        odv[jb],
                        outv[:, jb],
                        idxs[:],
                        prepare_only=True,
                        sem=sem,
                        queue_num=qn,
                    ).ins

            # ---- input DMAs (all SP HWDGE, wire order = emission order) ---
            HPX = 1024 + 8 * NL  # piS + d<4 half of X
            if getattr(sys.modules[__name__], "PXT_SPLIT3", False):
                HP1 = 1024 + 4 * NL  # piS + d0,d1
                nc.sync.dma_start(pxt[:, 0:HP1], pxt_d[:, 0:HP1])
                nc.sync.dma_start(pxt[:, HP1:HPX], pxt_d[:, HP1:HPX])
            else:
                nc.sync.dma_start(pxt[:, 0:HPX], pxt_d[:, 0:HPX])
            nc.sync.dma_start(pxt[:, HPX:PXT], pxt_d[:, HPX:PXT])
            nc.sync.dma_start(aux[:], aux_d[:, :])
            ytldv = ytl_d.rearrange("l (jb x) -> l jb x", jb=4)
            ytlsv = ytl.rearrange("l (jb x) -> l jb x", jb=4)
            for jb in range(4):
                nc.sync.dma_start(ytlsv[:, jb], ytldv[:, jb])

            # ---- PE p-state priming (dummy matmuls on scratch) ------------
            # pe_busy_start is pinned by the FIRST matmul and survives sub-us
            # idle gaps; a train of cheap dummies bridges until real work so
            # the 3us ramp elapses before the warp starts.
            if N_PRIME:
                dum = psm.tile([128, 256], f8, tag="dum")
                nc.vector.memset(dum[:], 1.0)
                dumv = dum.rearrange("l (t o) -> l t o", o=128)
                # all dummies hit ONE psum tile: the WAW chain is free on the
                # in-order PE, while buffer rotation would add sem waits.
                dps = pp.tile([1, 128], f32, tag="psO", bufs=3, name="dps")
                for i in range(N_PRIME):
                    nc.tensor.matmul(
                        dps[:], dumv[:, :, 0:1], dumv,
                        start=True, stop=True, perf_mode=DR,
                        skip_group_check=True,
                    )

            # ---- aug matmuls: psum group starters -------------------------
            # psO bufs=3 holds jb0-2; outps3 is allocated from the psW pool
            # (by the time its WAR resolves the warp psum cycle is drained).
            outps = [
                pp.tile([128, NL], f32, tag="psO", bufs=3, name=f"outps{jb}")
                for jb in range(3)
            ]

            def aug(jb):
                nc.tensor.matmul(
                    outps[jb][:],
                    augL[:, jb * 128 : (jb + 1) * 128],
                    augR,
                    start=True, stop=False, skip_group_check=True,
                )

            # ---- warp (PE) + contiguous evacs -----------------------------
            # 32 slot-matmuls of 144 cols, grouped 3-per-psum-tile (432 cols)
            # so each evac is one contiguous [128, 432] copy into xwt.
            # Group g covers units [3g, 3g+3), unit u = kc*2 + s.
            # Emission order: groups that only need the first pxt DMA first.
            xwt = pw.tile([128, 16 * NL], f8, tag="xwt")
            units = [(u // 2 // 8, (u // 2) % 8, u % 2) for u in range(32)]
            if getattr(sys.modules[__name__], "G2", False):
                gorder = [0, 1, 6, 7, 2, 3, 8, 9, 4, 5, 10]
            else:
                gorder = [0, 1, 6, 7, 2, 3, 4, 5, 8, 9, 10]

            # warp psum tiles cycle through BOTH pools (the jb psums are idle
            # until the augs, which anyway wait for the warp to drain) -- 8
            # effective slots instead of 5 keeps the WAR loop off the path.
            WTAG = ["psW", "psW", "psW", "psW", "psW", "psO", "psO", "psO",
                    "psW", "psW", "psW"]

            caps = (capA, capB)
            uoff = [0]
            for u in range(32):
                uoff.append(uoff[-1] + caps[u % 2])

            def warp(ei, g):
                u0 = 3 * g
                grp = units[u0 : u0 + 3]
                gw = uoff[u0 + len(grp)] - uoff[u0]
                w = pp.tile(
                    [128, gw], f32, tag=WTAG[ei],
                    bufs=5 if WTAG[ei] == "psW" else 3, name=f"xw{g}"
                )
                c0 = 0
                for i, (pc, d, s) in enumerate(grp):
                    so = s * capA
                    nc.tensor.matmul(
                        w[:, c0 : c0 + caps[s]],
                        piSv[:, s, :, pc * 128 : (pc + 1) * 128],
                        xt2v[:, d, :, so : so + caps[s]],
                        start=True, stop=True, perf_mode=DR,
                        skip_group_check=True,
                    )
                    c0 += caps[s]
                _copy(nc, XW_EVAC[ei],
                      xwt[:, uoff[u0] : uoff[u0] + gw], w[:])

            for ei, g in enumerate(gorder):
                warp(ei, g)
            outps.append(pp.tile([128, NL], f32, tag="psW", bufs=5, name="outps3"))
            for jb in range(4):
                aug(jb)
            xwtv = xwt.rearrange("l (kc n) -> l kc n", kc=16)

            # ---- C3: 8 DR passes per j-block, ordered by when the warp
            # groups covering each kc-pair land (see gorder). Rounds are
            # interleaved across j-blocks so only the last two rounds sit
            # behind the final warp evacs; jb3's early rounds slot in after
            # its (last) ytl chunk arrives.
            if getattr(sys.modules[__name__], "G2", False):
                korder = [0, 5, 1, 2, 6, 3, 4, 7]
            else:
                korder = [0, 5, 1, 2, 3, 4, 6, 7]

            def c3_pass(jb, ki):
                k = korder[ki]
                nc.tensor.matmul(
                    outps[jb][:],
                    ytlv[:, jb, 2 * k : 2 * k + 2, :],
                    xwtv[:, 2 * k : 2 * k + 2, :],
                    start=False, stop=(ki == 7), perf_mode=DR,
                    skip_group_check=True,
                )

            def c3_finish(jb):
                if jb == 3 and getattr(sys.modules[__name__], "JB3_SPLIT", False):
                    evs = [
                        _copy(nc, "a", outsb[:, jb * NL : jb * NL + capA],
                              outps[jb][:, 0:capA]),
                        _copy(nc, "d", outsb[:, jb * NL + capA : (jb + 1) * NL],
                              outps[jb][:, capA:NL]),
                    ]
                else:
                    evs = [_copy(nc, OUT_EVAC[jb],
                                 outsb[:, jb * NL : (jb + 1) * NL], outps[jb][:])]
                if jb in wb_sems:
                    # Drop the bogus WAR edge evac->prep (Tile attributes the
                    # prep's deferred outsb read to DMA completion, which
                    # would deadlock against the evac that PRODUCES the data).
                    # Real ordering: evac -> (sync dep) -> trigger -> DMA read.
                    qn = sorted(WB_JBS).index(jb)
                    dep = _br.InstructionNameOrderedSet()
                    for ev in evs:
                        ev.ins.remove_dependency(wb_prep[jb].name)
                        dep.add(ev.ins.name)
                    trg = nc.gpsimd.trigger_dma(count=None, queue_num=qn)
                    trg.ins.add_sync_dependencies_from(dep)
                    wb_trg[jb] = trg.ins
                else:
                    nc.sync.dma_start(
                        out_d[jb * 128 : (jb + 1) * 128, :],
                        outsb[:, jb * NL : (jb + 1) * NL],
                    )

            order = getattr(sys.modules[__name__], "C3_ORDER", 0)
            if order == 0:
                for ki in range(8):
                    for jb in range(3):
                        c3_pass(jb, ki)
                for jb in range(3):
                    c3_finish(jb)
                for ki in range(8):
                    c3_pass(3, ki)
                c3_finish(3)
            elif order == 1:
                # jb3's early rounds slot between jb0-2's finishes
                for ki in range(8):
                    for jb in range(3):
                        c3_pass(jb, ki)
                for ki in range(6):
                    c3_pass(3, ki)
                for jb in range(3):
                    c3_finish(jb)
                for ki in (6, 7):
                    c3_pass(3, ki)
                c3_finish(3)
            else:
                # jb0 drains fully first so its evac/trigger leave earliest
                for ki in range(8):
                    c3_pass(0, ki)
                c3_finish(0)
                for ki in range(8):
                    for jb in (1, 2):
                        c3_pass(jb, ki)
                for jb in (1, 2):
                    c3_finish(jb)
                for ki in range(8):
                    c3_pass(3, ki)
                c3_finish(3)

            # end-of-kernel: hold Pool until every writeback DMA completed
            # (replaces the DMASW lane waits stripped below, which the
            # timeline scheduler cannot satisfy for user-sem'd preps).
            # All completion waits anchor AFTER the LAST trigger: a wait
            # placed between triggers would serialize them by ~1us each.
            last_jb = max(WB_JBS) if nwb else None
            wge_names = {}
            for jb in sorted(WB_JBS):
                wge = nc.gpsimd.wait_ge(wb_sems[jb], 16)
                wge_names[jb] = wge.ins.name
                dep = _br.InstructionNameOrderedSet()
                dep.add(wb_trg[jb].name)
                dep.add(wb_trg[last_jb].name)
                wge.ins.add_sync_dependencies_from(dep)
            for jb in sorted(WB_JBS):
                desc = _br.InstructionNameOrderedSet()
                desc.add(wge_names[jb])
                if jb == last_jb:
                    for j2 in sorted(WB_JBS):
                        desc.add(wge_names[j2])
                wb_trg[jb].descendants = desc

    if nwb:
        for b in nc.m.functions[0].blocks:
            for i in b.instructions:
                si = i.sync_info
                if si is None:
                    continue
                ws = list(si.on_wait)
                if any("DMASW" in str(w) for w in ws):
                    si.on_wait = [w for w in ws if "DMASW" not in str(w)]

    nc.compile()
    return nc


def kernel(X, Y, pi_dtw, classes):
    import ml_dtypes
    from concourse.bass_utils import run_bass_kernel_spmd

    f8 = ml_dtypes.float8_e4m3
    X = np.ascontiguousarray(np.asarray(X, dtype=np.float32))
    Y = np.ascontiguousarray(np.asarray(Y, dtype=np.float32))
    pi_dtw = np.ascontiguousarray(np.asarray(pi_dtw, dtype=np.float32))
    classes = np.asarray(classes).astype(np.int64)

    counts = np.bincount(classes, minlength=C)
    # slot A holds the larger class of each pair, slot B the smaller, so the
    # SPMD-shared caps (and with them every evac/pass column count) shrink.
    pairs = [(2 * r, 2 * r + 1) for r in range(4)]
    slots = [(a, b) if counts[a] >= counts[b] else (b, a) for a, b in pairs]
    rup = lambda v, m: int(-(-int(v) // m) * m)
    capA = rup(max(counts[a] for a, b in slots), 16)
    capB = rup(max(counts[b] for a, b in slots), 4)
    NL = capA + capB

    if (capA, capB) not in _cache:
        _cache[(capA, capB)] = _build(capA, capB)
    nc = _cache[(capA, capB)]

    idx = [np.nonzero(classes == c)[0] for c in range(C)]

    # bias terms (host): row/col norms contracted with the pi sums
    qfull = (Y * Y).sum(axis=2)          # [NY, TP]
    rfull = (X * X).sum(axis=2)          # [N, T]
    colsum = pi_dtw.sum(axis=1)          # [C, TP]
    rowsum = pi_dtw.sum(axis=2)          # [C, T]
    C2 = qfull @ colsum.T                # [NY, C]
    C1 = (rfull * rowsum[classes]).sum(axis=1)  # [N]

    # per j-half: ytl (-2Y, [p_in, jb, pc, d, jj])
    ytls = []
    for cj in range(2):
        Yh = -2.0 * Y[cj * NYL : (cj + 1) * NYL]
        B = Yh.reshape(4, 128, 2, 128, D).transpose(3, 0, 2, 4, 1)
        ytls.append(np.ascontiguousarray(B.reshape(128, 16 * NYL)).astype(f8))

    in_maps = []
    for r in range(4):
        ca, cb = slots[r]
        Xp = np.zeros((NL, T, D), dtype=np.float32)
        Xp[0 : counts[ca]] = X[idx[ca]]
        Xp[capA : capA + counts[cb]] = X[idx[cb]]
        # [t_in, d, tc, n]
        xt2 = Xp.reshape(NL, 2, 128, D).transpose(2, 3, 1, 0).reshape(128, 16 * NL)

        P = pi_dtw[[ca, cb]]
        pis = P.reshape(2, 2, 128, 256).transpose(2, 0, 1, 3).reshape(128, 1024)
        pxt = np.ascontiguousarray(
            np.concatenate([pis, xt2], axis=1)
        ).astype(f8)

        c1c = np.zeros(NL, dtype=np.float32)
        c1c[0 : counts[ca]] = C1[idx[ca]]
        c1c[capA : capA + counts[cb]] = C1[idx[cb]]

        for cj in range(2):
            aux = np.zeros((4, NYL + NL + 16), dtype=np.float16)
            aux[0, 0:NYL] = C2[cj * NYL : (cj + 1) * NYL, ca]
            aux[1, 0:NYL] = C2[cj * NYL : (cj + 1) * NYL, cb]
            aux[2, 0:NYL] = 1.0
            aux[0, NYL : NYL + counts[ca]] = 1.0  # indA
            aux[1, NYL + capA : NYL + capA + counts[cb]] = 1.0  # indB
            aux[2, NYL : NYL + NL] = c1c
            in_maps.append(
                {"pxt": pxt, "ytl": ytls[cj], "aux": aux}
            )

    res = run_bass_kernel_spmd(nc, in_maps, core_ids=list(range(NCORES)))

    out = np.empty((N, NY), dtype=np.float32)
    jr = [np.arange(0, NYL), np.arange(NYL, NY)]
    for r in range(4):
        ca, cb = slots[r]
        for cj in range(2):
            blk = np.asarray(res.results[2 * r + cj]["out"]).astype(np.float32)
            out[np.ix_(idx[ca], jr[cj])] = blk[:, 0 : counts[ca]].T
            out[np.ix_(idx[cb], jr[cj])] = blk[:, capA : capA + counts[cb]].T
    return out


# revision 80
# speedup vs baseline: 1.0224x; 1.0071x over previous
"""Trainium2 Bass kernel for the CNN-MAD per-class DTW transport cost.

Math (reference):
  mat_cost[n, j] = C1[n] + C2[c_n, j] - 2*C3[n, j],  c_n = classes[n]
    C1[n]    = sum_t rowsum[c_n, t] * r[n,t],   r[n,t] = sum_d X[n,t,d]^2
    C2[c, j] = sum_p colsum[c, p]  * q[j,p],    q[j,p] = sum_d Y[j,p,d]^2
    C3[n, j] = sum_{p,d} XW[n,p,d] * Y[j,p,d],  XW = pi_c.T @ X (warp)

Sharding: 4x2 grid. Core (rr, cj) owns the samples of classes {2rr, 2rr+1}
and the j-half [512*cj, 512*(cj+1)).  The larger class of each pair goes
to slot A, the smaller to slot B; the SPMD-shared caps (capA, capB) are
the max slot sizes over cores (capB unrounded beyond 4), so NL=capA+capB
carries minimal padding.  One SPMD program for all 8 cores; per-core
class structure enters only through data.  The two big contractions (the
DTW warp and the X~Y inner-product field) run on the PE at fp8 DoubleRow
rate; the tiny bias terms C1/C2 (rank-1 row/col corrections) are
host-precomputed and enter each output psum through one rank-3 fp16
augmentation matmul per j-block:
  - warp XW = piS.T @ X, psum evac'd as a pure contiguous copy
    ((d,n)-major layout, -2 prefolded into the shipped Y).
  - C3 flipped to [j-partition, n-free] psum orientation: 4 j-blocks of
    128, 8 DR passes each over k=(p,d); cost scales with n=NL not NY.
  - outputs leave via SWDGE prepare/trigger writebacks (one queue per
    j-block): descriptors are generated early on Pool, each trigger
    fires right after its block's evac, so the post-compute tail is
    trigger+transfer+sem instead of a full HWDGE dispatch chain.
  - a train of cheap dummy matmuls pins pe_busy_start early so the 3us
    PE p-state ramp elapses before the real matmuls start.
"""

import sys

sys.path.insert(0, "/opt/trn_rl_repo")

import numpy as np

N, NY, T, TP, D, C = 1024, 1024, 256, 256, 8, 8
NCORES = 8
NYL = 512  # j columns per core

_cache = {}

# Engine per warp-psum evac, by emission index (a=ACT, d=DVE).
# Pool/GPSIMD cannot read PSUM on real hardware, so only ACT and DVE may
# evacuate psum tiles; Pool carries the writeback preps and triggers.
XW_EVAC = ["d", "d", "d", "a", "d", "a", "a", "a", "d", "a", "d"]
OUT_EVAC = ("a", "d", "a", "d")
N_PRIME = 52  # PE p-state priming matmuls (0 = off)
WB_JBS = (0, 1, 2, 3)  # j-blocks whose output goes via prepare/trigger writeback


def _copy(nc, eng, dst, src):
    if eng == "a":
        return nc.scalar.mul(dst, src, 1.0)
    elif eng == "d":
        return nc.vector.tensor_copy(dst, src)
    else:
        return nc.gpsimd.tensor_copy(dst, src)


def _build(capA, capB):
    import bass_rust as _br
    import concourse.bacc as bacc
    import concourse.mybir as mybir
    import concourse.tile as tile

    f8 = mybir.dt.float8e4
    bf = mybir.dt.bfloat16
    f16 = mybir.dt.float16
    f32 = mybir.dt.float32
    i32 = mybir.dt.int32
    DR = mybir.MatmulPerfMode.DoubleRow
    NL = capA + capB

    nwb = len(WB_JBS)
    nc = bacc.Bacc(
        "TRN2",
        target_bir_lowering=False,
        debug=False,
        num_devices=NCORES,
        num_swdge_queues=max(1, nwb),
    )

    # pxt = piS | X in (d, tc, n) layout: one contiguous DMA covers piS and
    # the first-half (d<4) warp operand, so the warp starts one transfer in.
    PXT = 1024 + 16 * NL
    pxt_d = nc.dram_tensor("pxt", [128, PXT], f8, kind="ExternalInput")
    ytl_d = nc.dram_tensor("ytl", [128, 16 * NYL], f8, kind="ExternalInput")
    aux_d = nc.dram_tensor("aux", [4, NYL + NL + 16], f16, kind="ExternalInput")
    out_d = nc.dram_tensor("out", [NYL, NL], bf, kind="ExternalOutput")

    with tile.TileContext(nc) as tc:
        with (
            tc.tile_pool(name="io", bufs=1) as pio,
            tc.tile_pool(name="work", bufs=1) as pw,
            tc.tile_pool(name="small", bufs=1) as psm,
            tc.tile_pool(name="ps", bufs=1, space="PSUM") as pp,
        ):
            pxt = pio.tile([128, PXT], f8, tag="pxt")
            ytl = pio.tile([128, 16 * NYL], f8, tag="ytl")
            aux = psm.tile([4, NYL + NL + 16], f16, tag="aux")
            outsb = pw.tile([128, 4 * NL], bf, tag="outsb")

            piSv = pxt[:, 0:1024].rearrange("l (c t p) -> l c t p", c=2, t=2)
            xt2v = pxt[:, 1024:PXT].rearrange("l (d t n) -> l d t n", d=8, t=2)
            ytlv = ytl.rearrange("l (jb kc j) -> l jb kc j", jb=4, kc=16)

            augL = aux[0:3, 0:NYL]            # [c2A | c2B | ones] over j
            augR = aux[0:3, NYL : NYL + NL]   # [indA | indB | c1c] over n

            # ---- writeback preps (descriptor gen; data read at trigger) ---
            wb_sems, wb_prep, wb_trg = {}, {}, {}
            if nwb:
                idxs = psm.tile([128, 2], i32, tag="wbidx")
                nc.gpsimd.memset(idxs[:], 0)
                outv = outsb.rearrange("j (jb o b n) -> j jb o b n", jb=4, o=1, b=2)
                odv = out_d.rearrange("(jb j o) (b n) -> jb b j o n", jb=4, o=1, b=2)
                for jb in sorted(WB_JBS):
                    qn = sorted(WB_JBS).index(jb)
                    sem = nc.alloc_semaphore(f"wbdma{jb}")
                    wb_sems[jb] = sem
                    wb_prep[jb] = nc.gpsimd.kv_writeback(
                        odv[jb],
                        outv[:, jb],
                        idxs[:],
                        prepare_only=True,
                        sem=sem,
                        queue_num=qn,
                    ).ins

            # ---- input DMAs (all SP HWDGE, wire order = emission order) ---
            HPX = 1024 + 8 * NL  # piS + d<4 half of X
            if getattr(sys.modules[__name__], "PXT_SPLIT3", False):
                HP1 = 1024 + 4 * NL  # piS + d0,d1
                nc.sync.dma_start(pxt[:, 0:HP1], pxt_d[:, 0:HP1])
                nc.sync.dma_start(pxt[:, HP1:HPX], pxt_d[:, HP1:HPX])
            else:
                nc.sync.dma_start(pxt[:, 0:HPX], pxt_d[:, 0:HPX])
            nc.sync.dma_start(pxt[:, HPX:PXT], pxt_d[:, HPX:PXT])
            nc.sync.dma_start(aux[:], aux_d[:, :])
            ytldv = ytl_d.rearrange("l (jb x) -> l jb x", jb=4)
            ytlsv = ytl.rearrange("l (jb x) -> l jb x", jb=4)
            for jb in range(4):
                nc.sync.dma_start(ytlsv[:, jb], ytldv[:, jb])

            # ---- PE p-state priming (dummy matmuls on scratch) ------------
            # pe_busy_start is pinned by the FIRST matmul and survives sub-us
            # idle gaps; a train of cheap dummies bridges until real work so
            # the 3us ramp elapses before the warp starts.
            if N_PRIME:
                dum = psm.tile([128, 256], f8, tag="dum")
                nc.vector.memset(dum[:], 1.0)
                dumv = dum.rearrange("l (t o) -> l t o", o=128)
                # all dummies hit ONE psum tile: the WAW chain is free on the
                # in-order PE, while buffer rotation would add sem waits.
                dps = pp.tile([1, 128], f32, tag="psO", bufs=3, name="dps")
                for i in range(N_PRIME):
                    nc.tensor.matmul(
                        dps[:], dumv[:, :, 0:1], dumv,
                        start=True, stop=True, perf_mode=DR,
                        skip_group_check=True,
                    )

            # ---- aug matmuls: psum group starters -------------------------
            # psO bufs=3 holds jb0-2; outps3 is allocated from the psW pool
            # (by the time its WAR resolves the warp psum cycle is drained).
            outps = [
                pp.tile([128, NL], f32, tag="psO", bufs=3, name=f"outps{jb}")
                for jb in range(3)
            ]

            def aug(jb):
                nc.tensor.matmul(
                    outps[jb][:],
                    augL[:, jb * 128 : (jb + 1) * 128],
                    augR,
                    start=True, stop=False, skip_group_check=True,
                )

            # ---- warp (PE) + contiguous evacs -----------------------------
            # 32 slot-matmuls of 144 cols, grouped 3-per-psum-tile (432 cols)
            # so each evac is one contiguous [128, 432] copy into xwt.
            # Group g covers units [3g, 3g+3), unit u = kc*2 + s.
            # Emission order: groups that only need the first pxt DMA first.
            xwt = pw.tile([128, 16 * NL], f8, tag="xwt")
            units = [(u // 2 // 8, (u // 2) % 8, u % 2) for u in range(32)]
            if getattr(sys.modules[__name__], "G2", False):
                gorder = [0, 1, 6, 7, 2, 3, 8, 9, 4, 5, 10]
            else:
                gorder = [0, 1, 6, 7, 2, 3, 4, 5, 8, 9, 10]

            # warp psum tiles cycle through BOTH pools (the jb psums are idle
            # until the augs, which anyway wait for the warp to drain) -- 8
            # effective slots instead of 5 keeps the WAR loop off the path.
            WTAG = ["psW", "psW", "psW", "psW", "psW", "psO", "psO", "psO",
                    "psW", "psW", "psW"]

            caps = (capA, capB)
            uoff = [0]
            for u in range(32):
                uoff.append(uoff[-1] + caps[u % 2])

            def warp(ei, g):
                u0 = 3 * g
                grp = units[u0 : u0 + 3]
                gw = uoff[u0 + len(grp)] - uoff[u0]
                w = pp.tile(
                    [128, gw], f32, tag=WTAG[ei],
                    bufs=5 if WTAG[ei] == "psW" else 3, name=f"xw{g}"
                )
                c0 = 0
                for i, (pc, d, s) in enumerate(grp):
                    so = s * capA
                    nc.tensor.matmul(
                        w[:, c0 : c0 + caps[s]],
                        piSv[:, s, :, pc * 128 : (pc + 1) * 128],
                        xt2v[:, d, :, so : so + caps[s]],
                        start=True, stop=True, perf_mode=DR,
                        skip_group_check=True,
                    )
                    c0 += caps[s]
                _copy(nc, XW_EVAC[ei],
                      xwt[:, uoff[u0] : uoff[u0] + gw], w[:])

            for ei, g in enumerate(gorder):
                warp(ei, g)
            outps.append(pp.tile([128, NL], f32, tag="psW", bufs=5, name="outps3"))
            for jb in range(4):
                aug(jb)
            xwtv = xwt.rearrange("l (kc n) -> l kc n", kc=16)

            # ---- C3: 8 DR passes per j-block, ordered by when the warp
            # groups covering each kc-pair land (see gorder). Rounds are
            # interleaved across j-blocks so only the last two rounds sit
            # behind the final warp evacs; jb3's early rounds slot in after
            # its (last) ytl chunk arrives.
            if getattr(sys.modules[__name__], "G2", False):
                korder = [0, 5, 1, 2, 6, 3, 4, 7]
            else:
                korder = [0, 5, 1, 2, 3, 4, 6, 7]

            def c3_pass(jb, ki):
                k = korder[ki]
                nc.tensor.matmul(
                    outps[jb][:],
                    ytlv[:, jb, 2 * k : 2 * k + 2, :],
                    xwtv[:, 2 * k : 2 * k + 2, :],
                    start=False, stop=(ki == 7), perf_mode=DR,
                    skip_group_check=True,
                )

            def c3_finish(jb):
                if jb == 3 and getattr(sys.modules[__name__], "JB3_SPLIT", False):
                    evs = [
                        _copy(nc, "a", outsb[:, jb * NL : jb * NL + capA],
                              outps[jb][:, 0:capA]),
                        _copy(nc, "d", outsb[:, jb * NL + capA : (jb + 1) * NL],
                              outps[jb][:, capA:NL]),
                    ]
                else:
                    evs = [_copy(nc, OUT_EVAC[jb],
                                 outsb[:, jb * NL : (jb + 1) * NL], outps[jb][:])]
                if jb in wb_sems:
                    # Drop the bogus WAR edge evac->prep (Tile attributes the
                    # prep's deferred outsb read to DMA completion, which
                    # would deadlock against the evac that PRODUCES the data).
                    # Real ordering: evac -> (sync dep) -> trigger -> DMA read.
                    qn = sorted(WB_JBS).index(jb)
                    dep = _br.InstructionNameOrderedSet()
                    for ev in evs:
                        ev.ins.remove_dependency(wb_prep[jb].name)
                        dep.add(ev.ins.name)
                    trg = nc.gpsimd.trigger_dma(count=None, queue_num=qn)
                    trg.ins.add_sync_dependencies_from(dep)
                    wb_trg[jb] = trg.ins
                else:
                    nc.sync.dma_start(
                        out_d[jb * 128 : (jb + 1) * 128, :],
                        outsb[:, jb * NL : (jb + 1) * NL],
                    )

            order = getattr(sys.modules[__name__], "C3_ORDER", 0)
            if order == 0:
                for ki in range(8):
                    for jb in range(3):
                        c3_pass(jb, ki)
                for jb in range(3):
                    c3_finish(jb)
                for ki in range(8):
                    c3_pass(3, ki)
                c3_finish(3)
            elif order == 1:
                # jb3's early rounds slot between jb0-2's finishes
                for ki in range(8):
                    for jb in range(3):
                        c3_pass(jb, ki)
                for ki in range(6):
                    c3_pass(3, ki)
                for jb in range(3):
                    c3_finish(jb)
                for ki in (6, 7):
                    c3_pass(3, ki)
                c3_finish(3)
            else:
                # jb0 drains fully first so its evac/trigger leave earliest
                for ki in range(8):
                    c3_pass(0, ki)
                c3_finish(0)
                for ki in range(8):
                    for jb in (1, 2):
                        c3_pass(jb, ki)
                for jb in (1, 2):
                    c3_finish(jb)
                for ki in range(8):
                    c3_pass(3, ki)
                c3_finish(3)

            # end-of-kernel: hold Pool until every writeback DMA completed
            # (replaces the DMASW lane waits stripped below, which the
            # timeline scheduler cannot satisfy for user-sem'd preps).
            # All completion waits anchor AFTER the LAST trigger: a wait
            # placed between triggers would serialize them by ~1us each.
            last_jb = max(WB_JBS) if nwb else None
            wge_names = {}
            for jb in sorted(WB_JBS):
                wge = nc.gpsimd.wait_ge(wb_sems[jb], 16)
                wge_names[jb] = wge.ins.name
                dep = _br.InstructionNameOrderedSet()
                dep.add(wb_trg[jb].name)
                dep.add(wb_trg[last_jb].name)
                wge.ins.add_sync_dependencies_from(dep)
            for jb in sorted(WB_JBS):
                desc = _br.InstructionNameOrderedSet()
                desc.add(wge_names[jb])
                if jb == last_jb:
                    for j2 in sorted(WB_JBS):
                        desc.add(wge_names[j2])
                wb_trg[jb].descendants = desc

    if nwb:
        for b in nc.m.functions[0].blocks:
            for i in b.instructions:
                si = i.sync_info
                if si is None:
                    continue
                ws = list(si.on_wait)
                if any("DMASW" in str(w) for w in ws):
                    si.on_wait = [w for w in ws if "DMASW" not in str(w)]

    nc.compile()
    return nc


def kernel(X, Y, pi_dtw, classes):
    import ml_dtypes
    from concourse.bass_utils import run_bass_kernel_spmd

    f8 = ml_dtypes.float8_e4m3
    X = np.ascontiguousarray(np.asarray(X, dtype=np.float32))
    Y = np.ascontiguousarray(np.asarray(Y, dtype=np.float32))
    pi_dtw = np.ascontiguousarray(np.asarray(pi_dtw, dtype=np.float32))
    classes = np.asarray(classes).astype(np.int64)

    counts = np.bincount(classes, minlength=C)
    # slot A holds the larger class of each pair, slot B the smaller, so the
    # SPMD-shared caps (and with them every evac/pass column count) shrink.
    pairs = [(2 * r, 2 * r + 1) for r in range(4)]
    slots = [(a, b) if counts[a] >= counts[b] else (b, a) for a, b in pairs]
    rup = lambda v, m: int(-(-int(v) // m) * m)
    capA = rup(max(counts[a] for a, b in slots), 16)
    capB = rup(max(counts[b] for a, b in slots), 4)
    NL = capA + capB

    if (capA, capB) not in _cache:
        _cache[(capA, capB)] = _build(capA, capB)
    nc = _cache[(capA, capB)]

    idx = [np.nonzero(classes == c)[0] for c in range(C)]

    # bias terms (host): row/col norms contracted with the pi sums
    qfull = (Y * Y).sum(axis=2)          # [NY, TP]
    rfull = (X * X).sum(axis=2)          # [N, T]
    colsum = pi_dtw.sum(axis=1)          # [C, TP]
    rowsum = pi_dtw.sum(axis=2)          # [C, T]
    C2 = qfull @ colsum.T                # [NY, C]
    C1 = (rfull * rowsum[classes]).sum(axis=1)  # [N]

    # per j-half: ytl (-2Y, [p_in, jb, pc, d, jj])
    ytls = []
    for cj in range(2):
        Yh = -2.0 * Y[cj * NYL : (cj + 1) * NYL]
        B = Yh.reshape(4, 128, 2, 128, D).transpose(3, 0, 2, 4, 1)
        ytls.append(np.ascontiguousarray(B.reshape(128, 16 * NYL)).astype(f8))

    in_maps = []
    for r in range(4):
        ca, cb = slots[r]
        Xp = np.zeros((NL, T, D), dtype=np.float32)
        Xp[0 : counts[ca]] = X[idx[ca]]
        Xp[capA : capA + counts[cb]] = X[idx[cb]]
        # [t_in, d, tc, n]
        xt2 = Xp.reshape(NL, 2, 128, D).transpose(2, 3, 1, 0).reshape(128, 16 * NL)

        P = pi_dtw[[ca, cb]]
        pis = P.reshape(2, 2, 128, 256).transpose(2, 0, 1, 3).reshape(128, 1024)
        pxt = np.ascontiguousarray(
            np.concatenate([pis, xt2], axis=1)
        ).astype(f8)

        c1c = np.zeros(NL, dtype=np.float32)
        c1c[0 : counts[ca]] = C1[idx[ca]]
        c1c[capA : capA + counts[cb]] = C1[idx[cb]]

        for cj in range(2):
            aux = np.zeros((4, NYL + NL + 16), dtype=np.float16)
            aux[0, 0:NYL] = C2[cj * NYL : (cj + 1) * NYL, ca]
            aux[1, 0:NYL] = C2[cj * NYL : (cj + 1) * NYL, cb]
            aux[2, 0:NYL] = 1.0
            aux[0, NYL : NYL + counts[ca]] = 1.0  # indA
            aux[1, NYL + capA : NYL + capA + counts[cb]] = 1.0  # indB
            aux[2, NYL : NYL + NL] = c1c
            in_maps.append(
                {"pxt": pxt, "ytl": ytls[cj], "aux": aux}
            )

    res = run_bass_kernel_spmd(nc, in_maps, core_ids=list(range(NCORES)))

    out = np.empty((N, NY), dtype=np.float32)
    jr = [np.arange(0, NYL), np.arange(NYL, NY)]
    for r in range(4):
        ca, cb = slots[r]
        for cj in range(2):
            blk = np.asarray(res.results[2 * r + cj]["out"]).astype(np.float32)
            out[np.ix_(idx[ca], jr[cj])] = blk[:, 0 : counts[ca]].T
            out[np.ix_(idx[cb], jr[cj])] = blk[:, capA : capA + counts[cb]].T
    return out


# revision 81
# speedup vs baseline: 1.0243x; 1.0019x over previous
"""Trainium2 Bass kernel for the CNN-MAD per-class DTW transport cost.

Math (reference):
  mat_cost[n, j] = C1[n] + C2[c_n, j] - 2*C3[n, j],  c_n = classes[n]
    C1[n]    = sum_t rowsum[c_n, t] * r[n,t],   r[n,t] = sum_d X[n,t,d]^2
    C2[c, j] = sum_p colsum[c, p]  * q[j,p],    q[j,p] = sum_d Y[j,p,d]^2
    C3[n, j] = sum_{p,d} XW[n,p,d] * Y[j,p,d],  XW = pi_c.T @ X (warp)

Sharding: 4x2 grid. Core (rr, cj) owns the samples of classes {2rr, 2rr+1}
and the j-half [512*cj, 512*(cj+1)).  The larger class of each pair goes
to slot A, the smaller to slot B; the SPMD-shared caps (capA, capB) are
the max slot sizes over cores (capB unrounded beyond 4), so NL=capA+capB
carries minimal padding.  One SPMD program for all 8 cores; per-core
class structure enters only through data.  The two big contractions (the
DTW warp and the X~Y inner-product field) run on the PE at fp8 DoubleRow
rate; the tiny bias terms C1/C2 (rank-1 row/col corrections) are
host-precomputed and enter each output psum through one rank-3 fp16
augmentation matmul per j-block:
  - warp XW = piS.T @ X, psum evac'd as a pure contiguous copy
    ((d,n)-major layout, -2 prefolded into the shipped Y).
  - C3 flipped to [j-partition, n-free] psum orientation: 4 j-blocks of
    128, 8 DR passes each over k=(p,d); cost scales with n=NL not NY.
  - outputs leave via SWDGE prepare/trigger writebacks (one queue per
    j-block): descriptors are generated early on Pool, each trigger
    fires right after its block's evac, so the post-compute tail is
    trigger+transfer+sem instead of a full HWDGE dispatch chain.
  - a train of cheap dummy matmuls pins pe_busy_start early so the 3us
    PE p-state ramp elapses before the real matmuls start.
"""

import sys

sys.path.insert(0, "/opt/trn_rl_repo")

import numpy as np

N, NY, T, TP, D, C = 1024, 1024, 256, 256, 8, 8
NCORES = 8
NYL = 512  # j columns per core

_cache = {}

# Engine per warp-psum evac, by emission index (a=ACT, d=DVE).
# Pool/GPSIMD cannot read PSUM on real hardware, so only ACT and DVE may
# evacuate psum tiles; Pool carries the writeback preps and triggers.
XW_EVAC = ["d", "d", "a", "d", "d", "a", "a", "a", "d", "a", "d"]
OUT_EVAC = ("a", "d", "a", "d")
N_PRIME = 52  # PE p-state priming matmuls (0 = off)
WB_JBS = (0, 1, 2, 3)  # j-blocks whose output goes via prepare/trigger writeback


def _copy(nc, eng, dst, src):
    if eng == "a":
        return nc.scalar.mul(dst, src, 1.0)
    elif eng == "d":
        return nc.vector.tensor_copy(dst, src)
    else:
        return nc.gpsimd.tensor_copy(dst, src)


def _build(capA, capB):
    import bass_rust as _br
    import concourse.bacc as bacc
    import concourse.mybir as mybir
    import concourse.tile as tile

    f8 = mybir.dt.float8e4
    bf = mybir.dt.bfloat16
    f16 = mybir.dt.float16
    f32 = mybir.dt.float32
    i32 = mybir.dt.int32
    DR = mybir.MatmulPerfMode.DoubleRow
    NL = capA + capB

    nwb = len(WB_JBS)
    nc = bacc.Bacc(
        "TRN2",
        target_bir_lowering=False,
        debug=False,
        num_devices=NCORES,
        num_swdge_queues=max(1, nwb),
    )

    # pxt = piS | X in (d, tc, n) layout: one contiguous DMA covers piS and
    # the first-half (d<4) warp operand, so the warp starts one transfer in.
    PXT = 1024 + 16 * NL
    pxt_d = nc.dram_tensor("pxt", [128, PXT], f8, kind="ExternalInput")
    ytl_d = nc.dram_tensor("ytl", [128, 16 * NYL], f8, kind="ExternalInput")
    aux_d = nc.dram_tensor("aux", [4, NYL + NL + 16], f16, kind="ExternalInput")
    out_d = nc.dram_tensor("out", [NYL, NL], bf, kind="ExternalOutput")

    with tile.TileContext(nc) as tc:
        with (
            tc.tile_pool(name="io", bufs=1) as pio,
            tc.tile_pool(name="work", bufs=1) as pw,
            tc.tile_pool(name="small", bufs=1) as psm,
            tc.tile_pool(name="ps", bufs=1, space="PSUM") as pp,
        ):
            pxt = pio.tile([128, PXT], f8, tag="pxt")
            ytl = pio.tile([128, 16 * NYL], f8, tag="ytl")
            aux = psm.tile([4, NYL + NL + 16], f16, tag="aux")
            outsb = pw.tile([128, 4 * NL], bf, tag="outsb")

            piSv = pxt[:, 0:1024].rearrange("l (c t p) -> l c t p", c=2, t=2)
            xt2v = pxt[:, 1024:PXT].rearrange("l (d t n) -> l d t n", d=8, t=2)
            ytlv = ytl.rearrange("l (jb kc j) -> l jb kc j", jb=4, kc=16)

            augL = aux[0:3, 0:NYL]            # [c2A | c2B | ones] over j
            augR = aux[0:3, NYL : NYL + NL]   # [indA | indB | c1c] over n

            # ---- writeback preps (descriptor gen; data read at trigger) ---
            wb_sems, wb_prep, wb_trg = {}, {}, {}
            if nwb:
                idxs = psm.tile([128, 2], i32, tag="wbidx")
                nc.gpsimd.memset(idxs[:], 0)
                outv = outsb.rearrange("j (jb o b n) -> j jb o b n", jb=4, o=1, b=2)
                odv = out_d.rearrange("(jb j o) (b n) -> jb b j o n", jb=4, o=1, b=2)
                for jb in sorted(WB_JBS):
                    qn = sorted(WB_JBS).index(jb)
                    sem = nc.alloc_semaphore(f"wbdma{jb}")
                    wb_sems[jb] = sem
                    wb_prep[jb] = nc.gpsimd.kv_writeback(
                        odv[jb],
                        outv[:, jb],
                        idxs[:],
                        prepare_only=True,
                        sem=sem,
                        queue_num=qn,
                    ).ins

            # ---- input DMAs (all SP HWDGE, wire order = emission order) ---
            HPX = 1024 + 8 * NL  # piS + d<4 half of X
            if getattr(sys.modules[__name__], "PXT_SPLIT3", False):
                HP1 = 1024 + 4 * NL  # piS + d0,d1
                nc.sync.dma_start(pxt[:, 0:HP1], pxt_d[:, 0:HP1])
                nc.sync.dma_start(pxt[:, HP1:HPX], pxt_d[:, HP1:HPX])
            else:
                nc.sync.dma_start(pxt[:, 0:HPX], pxt_d[:, 0:HPX])
            nc.sync.dma_start(pxt[:, HPX:PXT], pxt_d[:, HPX:PXT])
            nc.sync.dma_start(aux[:], aux_d[:, :])
            ytldv = ytl_d.rearrange("l (jb x) -> l jb x", jb=4)
            ytlsv = ytl.rearrange("l (jb x) -> l jb x", jb=4)
            for jb in range(4):
                nc.sync.dma_start(ytlsv[:, jb], ytldv[:, jb])

            # ---- PE p-state priming (dummy matmuls on scratch) ------------
            # pe_busy_start is pinned by the FIRST matmul and survives sub-us
            # idle gaps; a train of cheap dummies bridges until real work so
            # the 3us ramp elapses before the warp starts.
            if N_PRIME:
                dum = psm.tile([128, 256], f8, tag="dum")
                nc.vector.memset(dum[:], 1.0)
                dumv = dum.rearrange("l (t o) -> l t o", o=128)
                # all dummies hit ONE psum tile: the WAW chain is free on the
                # in-order PE, while buffer rotation would add sem waits.
                dps = pp.tile([1, 128], f32, tag="psO", bufs=3, name="dps")
                for i in range(N_PRIME):
                    nc.tensor.matmul(
                        dps[:], dumv[:, :, 0:1], dumv,
                        start=True, stop=True, perf_mode=DR,
                        skip_group_check=True,
                    )

            # ---- aug matmuls: psum group starters -------------------------
            # psO bufs=3 holds jb0-2; outps3 is allocated from the psW pool
            # (by the time its WAR resolves the warp psum cycle is drained).
            outps = [
                pp.tile([128, NL], f32, tag="psO", bufs=3, name=f"outps{jb}")
                for jb in range(3)
            ]

            def aug(jb):
                nc.tensor.matmul(
                    outps[jb][:],
                    augL[:, jb * 128 : (jb + 1) * 128],
                    augR,
                    start=True, stop=False, skip_group_check=True,
                )

            # ---- warp (PE) + contiguous evacs -----------------------------
            # 32 slot-matmuls of 144 cols, grouped 3-per-psum-tile (432 cols)
            # so each evac is one contiguous [128, 432] copy into xwt.
            # Group g covers units [3g, 3g+3), unit u = kc*2 + s.
            # Emission order: groups that only need the first pxt DMA first.
            xwt = pw.tile([128, 16 * NL], f8, tag="xwt")
            units = [(u // 2 // 8, (u // 2) % 8, u % 2) for u in range(32)]
            if getattr(sys.modules[__name__], "G2", False):
                gorder = [0, 1, 6, 7, 2, 3, 8, 9, 4, 5, 10]
            else:
                gorder = [0, 1, 6, 7, 2, 3, 4, 5, 8, 9, 10]

            # warp psum tiles cycle through BOTH pools (the jb psums are idle
            # until the augs, which anyway wait for the warp to drain) -- 8
            # effective slots instead of 5 keeps the WAR loop off the path.
            WTAG = ["psW", "psW", "psW", "psW", "psW", "psO", "psO", "psO",
                    "psW", "psW", "psW"]

            caps = (capA, capB)
            uoff = [0]
            for u in range(32):
                uoff.append(uoff[-1] + caps[u % 2])

            def warp(ei, g):
                u0 = 3 * g
                grp = units[u0 : u0 + 3]
                gw = uoff[u0 + len(grp)] - uoff[u0]
                w = pp.tile(
                    [128, gw], f32, tag=WTAG[ei],
                    bufs=5 if WTAG[ei] == "psW" else 3, name=f"xw{g}"
                )
                c0 = 0
                for i, (pc, d, s) in enumerate(grp):
                    so = s * capA
                    nc.tensor.matmul(
                        w[:, c0 : c0 + caps[s]],
                        piSv[:, s, :, pc * 128 : (pc + 1) * 128],
                        xt2v[:, d, :, so : so + caps[s]],
                        start=True, stop=True, perf_mode=DR,
                        skip_group_check=True,
                    )
                    c0 += caps[s]
                _copy(nc, XW_EVAC[ei],
                      xwt[:, uoff[u0] : uoff[u0] + gw], w[:])

            for ei, g in enumerate(gorder):
                warp(ei, g)
            outps.append(pp.tile([128, NL], f32, tag="psW", bufs=5, name="outps3"))
            for jb in range(4):
                aug(jb)
            xwtv = xwt.rearrange("l (kc n) -> l kc n", kc=16)

            # ---- C3: 8 DR passes per j-block, ordered by when the warp
            # groups covering each kc-pair land (see gorder). Rounds are
            # interleaved across j-blocks so only the last two rounds sit
            # behind the final warp evacs; jb3's early rounds slot in after
            # its (last) ytl chunk arrives.
            if getattr(sys.modules[__name__], "G2", False):
                korder = [0, 5, 1, 2, 6, 3, 4, 7]
            else:
                korder = [0, 5, 1, 2, 3, 4, 6, 7]

            def c3_pass(jb, ki):
                k = korder[ki]
                nc.tensor.matmul(
                    outps[jb][:],
                    ytlv[:, jb, 2 * k : 2 * k + 2, :],
                    xwtv[:, 2 * k : 2 * k + 2, :],
                    start=False, stop=(ki == 7), perf_mode=DR,
                    skip_group_check=True,
                )

            def c3_finish(jb):
                if jb == 3 and getattr(sys.modules[__name__], "JB3_SPLIT", False):
                    evs = [
                        _copy(nc, "a", outsb[:, jb * NL : jb * NL + capA],
                              outps[jb][:, 0:capA]),
                        _copy(nc, "d", outsb[:, jb * NL + capA : (jb + 1) * NL],
                              outps[jb][:, capA:NL]),
                    ]
                else:
                    evs = [_copy(nc, OUT_EVAC[jb],
                                 outsb[:, jb * NL : (jb + 1) * NL], outps[jb][:])]
                if jb in wb_sems:
                    # Drop the bogus WAR edge evac->prep (Tile attributes the
                    # prep's deferred outsb read to DMA completion, which
                    # would deadlock against the evac that PRODUCES the data).
                    # Real ordering: evac -> (sync dep) -> trigger -> DMA read.
                    qn = sorted(WB_JBS).index(jb)
                    dep = _br.InstructionNameOrderedSet()
                    for ev in evs:
                        ev.ins.remove_dependency(wb_prep[jb].name)
                        dep.add(ev.ins.name)
                    trg = nc.gpsimd.trigger_dma(count=None, queue_num=qn)
                    trg.ins.add_sync_dependencies_from(dep)
                    wb_trg[jb] = trg.ins
                else:
                    nc.sync.dma_start(
                        out_d[jb * 128 : (jb + 1) * 128, :],
                        outsb[:, jb * NL : (jb + 1) * NL],
                    )

            order = getattr(sys.modules[__name__], "C3_ORDER", 0)
            if order == 0:
                for ki in range(8):
                    for jb in range(3):
                        c3_pass(jb, ki)
                for jb in range(3):
                    c3_finish(jb)
                for ki in range(8):
                    c3_pass(3, ki)
                c3_finish(3)
            elif order == 1:
                # jb3's early rounds slot between jb0-2's finishes
                for ki in range(8):
                    for jb in range(3):
                        c3_pass(jb, ki)
                for ki in range(6):
                    c3_pass(3, ki)
                for jb in range(3):
                    c3_finish(jb)
                for ki in (6, 7):
                    c3_pass(3, ki)
                c3_finish(3)
            else:
                # jb0 drains fully first so its evac/trigger leave earliest
                for ki in range(8):
                    c3_pass(0, ki)
                c3_finish(0)
                for ki in range(8):
                    for jb in (1, 2):
                        c3_pass(jb, ki)
                for jb in (1, 2):
                    c3_finish(jb)
                for ki in range(8):
                    c3_pass(3, ki)
                c3_finish(3)

            # end-of-kernel: hold Pool until every writeback DMA completed
            # (replaces the DMASW lane waits stripped below, which the
            # timeline scheduler cannot satisfy for user-sem'd preps).
            # All completion waits anchor AFTER the LAST trigger: a wait
            # placed between triggers would serialize them by ~1us each.
            last_jb = max(WB_JBS) if nwb else None
            wge_names = {}
            for jb in sorted(WB_JBS):
                wge = nc.gpsimd.wait_ge(wb_sems[jb], 16)
                wge_names[jb] = wge.ins.name
                dep = _br.InstructionNameOrderedSet()
                dep.add(wb_trg[jb].name)
                dep.add(wb_trg[last_jb].name)
                wge.ins.add_sync_dependencies_from(dep)
            for jb in sorted(WB_JBS):
                desc = _br.InstructionNameOrderedSet()
                desc.add(wge_names[jb])
                if jb == last_jb:
                    for j2 in sorted(WB_JBS):
                        desc.add(wge_names[j2])
                wb_trg[jb].descendants = desc

    if nwb:
        for b in nc.m.functions[0].blocks:
            for i in b.instructions:
                si = i.sync_info
                if si is None:
                    continue
                ws = list(si.on_wait)
                if any("DMASW" in str(w) for w in ws):
                    si.on_wait = [w for w in ws if "DMASW" not in str(w)]

    nc.compile()
    return nc


def kernel(X, Y, pi_dtw, classes):
    import ml_dtypes
    from concourse.bass_utils import run_bass_kernel_spmd

    f8 = ml_dtypes.float8_e4m3
    X = np.ascontiguousarray(np.asarray(X, dtype=np.float32))
    Y = np.ascontiguousarray(np.asarray(Y, dtype=np.float32))
    pi_dtw = np.ascontiguousarray(np.asarray(pi_dtw, dtype=np.float32))
    classes = np.asarray(classes).astype(np.int64)

    counts = np.bincount(classes, minlength=C)
    # slot A holds the larger class of each pair, slot B the smaller, so the
    # SPMD-shared caps (and with them every evac/pass column count) shrink.
    pairs = [(2 * r, 2 * r + 1) for r in range(4)]
    slots = [(a, b) if counts[a] >= counts[b] else (b, a) for a, b in pairs]
    rup = lambda v, m: int(-(-int(v) // m) * m)
    capA = rup(max(counts[a] for a, b in slots), 16)
    capB = rup(max(counts[b] for a, b in slots), 4)
    NL = capA + capB

    if (capA, capB) not in _cache:
        _cache[(capA, capB)] = _build(capA, capB)
    nc = _cache[(capA, capB)]

    idx = [np.nonzero(classes == c)[0] for c in range(C)]

    # bias terms (host): row/col norms contracted with the pi sums
    qfull = (Y * Y).sum(axis=2)          # [NY, TP]
    rfull = (X * X).sum(axis=2)          # [N, T]
    colsum = pi_dtw.sum(axis=1)          # [C, TP]
    rowsum = pi_dtw.sum(axis=2)          # [C, T]
    C2 = qfull @ colsum.T                # [NY, C]
    C1 = (rfull * rowsum[classes]).sum(axis=1)  # [N]

    # per j-half: ytl (-2Y, [p_in, jb, pc, d, jj])
    ytls = []
    for cj in range(2):
        Yh = -2.0 * Y[cj * NYL : (cj + 1) * NYL]
        B = Yh.reshape(4, 128, 2, 128, D).transpose(3, 0, 2, 4, 1)
        ytls.append(np.ascontiguousarray(B.reshape(128, 16 * NYL)).astype(f8))

    in_maps = []
    for r in range(4):
        ca, cb = slots[r]
        Xp = np.zeros((NL, T, D), dtype=np.float32)
        Xp[0 : counts[ca]] = X[idx[ca]]
        Xp[capA : capA + counts[cb]] = X[idx[cb]]
        # [t_in, d, tc, n]
        xt2 = Xp.reshape(NL, 2, 128, D).transpose(2, 3, 1, 0).reshape(128, 16 * NL)

        P = pi_dtw[[ca, cb]]
        pis = P.reshape(2, 2, 128, 256).transpose(2, 0, 1, 3).reshape(128, 1024)
        pxt = np.ascontiguousarray(
            np.concatenate([pis, xt2], axis=1)
        ).astype(f8)

        c1c = np.zeros(NL, dtype=np.float32)
        c1c[0 : counts[ca]] = C1[idx[ca]]
        c1c[capA : capA + counts[cb]] = C1[idx[cb]]

        for cj in range(2):
            aux = np.zeros((4, NYL + NL + 16), dtype=np.float16)
            aux[0, 0:NYL] = C2[cj * NYL : (cj + 1) * NYL, ca]
            aux[1, 0:NYL] = C2[cj * NYL : (cj + 1) * NYL, cb]
            aux[2, 0:NYL] = 1.0
            aux[0, NYL : NYL + counts[ca]] = 1.0  # indA
            aux[1, NYL + capA : NYL + capA + counts[cb]] = 1.0  # indB
            aux[2, NYL : NYL + NL] = c1c
            in_maps.append(
                {"pxt": pxt, "ytl": ytls[cj], "aux": aux}
            )

    res = run_bass_kernel_spmd(nc, in_maps, core_ids=list(range(NCORES)))

    out = np.empty((N, NY), dtype=np.float32)
    jr = [np.arange(0, NYL), np.arange(NYL, NY)]
    for r in range(4):
        ca, cb = slots[r]
        for cj in range(2):
            blk = np.asarray(res.results[2 * r + cj]["out"]).astype(np.float32)
            out[np.ix_(idx[ca], jr[cj])] = blk[:, 0 : counts[ca]].T
            out[np.ix_(idx[cb], jr[cj])] = blk[:, capA : capA + counts[cb]].T
    return out


# revision 82
# speedup vs baseline: 1.0252x; 1.0008x over previous
"""Trainium2 Bass kernel for the CNN-MAD per-class DTW transport cost.

Math (reference):
  mat_cost[n, j] = C1[n] + C2[c_n, j] - 2*C3[n, j],  c_n = classes[n]
    C1[n]    = sum_t rowsum[c_n, t] * r[n,t],   r[n,t] = sum_d X[n,t,d]^2
    C2[c, j] = sum_p colsum[c, p]  * q[j,p],    q[j,p] = sum_d Y[j,p,d]^2
    C3[n, j] = sum_{p,d} XW[n,p,d] * Y[j,p,d],  XW = pi_c.T @ X (warp)

Sharding: 4x2 grid. Core (rr, cj) owns the samples of classes {2rr, 2rr+1}
and the j-half [512*cj, 512*(cj+1)).  The larger class of each pair goes
to slot A, the smaller to slot B; the SPMD-shared caps (capA, capB) are
the max slot sizes over cores (capB unrounded beyond 4), so NL=capA+capB
carries minimal padding.  One SPMD program for all 8 cores; per-core
class structure enters only through data.  The two big contractions (the
DTW warp and the X~Y inner-product field) run on the PE at fp8 DoubleRow
rate; the tiny bias terms C1/C2 (rank-1 row/col corrections) are
host-precomputed and enter each output psum through one rank-3 fp16
augmentation matmul per j-block:
  - warp XW = piS.T @ X, psum evac'd as a pure contiguous copy
    ((d,n)-major layout, -2 prefolded into the shipped Y).
  - C3 flipped to [j-partition, n-free] psum orientation: 4 j-blocks of
    128, 8 DR passes each over k=(p,d); cost scales with n=NL not NY.
  - outputs leave via SWDGE prepare/trigger writebacks (one queue per
    j-block): descriptors are generated early on Pool, each trigger
    fires right after its block's evac, so the post-compute tail is
    trigger+transfer+sem instead of a full HWDGE dispatch chain.
  - a train of cheap dummy matmuls pins pe_busy_start early so the 3us
    PE p-state ramp elapses before the real matmuls start.
"""

import sys

sys.path.insert(0, "/opt/trn_rl_repo")

import numpy as np

N, NY, T, TP, D, C = 1024, 1024, 256, 256, 8, 8
NCORES = 8
NYL = 512  # j columns per core

_cache = {}

# Engine per warp-psum evac, by emission index (a=ACT, d=DVE).
# Pool/GPSIMD cannot read PSUM on real hardware, so only ACT and DVE may
# evacuate psum tiles; Pool carries the writeback preps and triggers.
XW_EVAC = ["d", "d", "a", "d", "d", "a", "a", "a", "d", "a", "d"]
OUT_EVAC = ("a", "d", "a", "d")
N_PRIME = 52  # PE p-state priming matmuls (0 = off)
WB_JBS = (0, 1, 2, 3)  # j-blocks whose output goes via prepare/trigger writeback


def _copy(nc, eng, dst, src):
    if eng == "a":
        return nc.scalar.mul(dst, src, 1.0)
    elif eng == "d":
        return nc.vector.tensor_copy(dst, src)
    else:
        return nc.gpsimd.tensor_copy(dst, src)


def _build(capA, capB):
    import bass_rust as _br
    import concourse.bacc as bacc
    import concourse.mybir as mybir
    import concourse.tile as tile

    f8 = mybir.dt.float8e4
    bf = mybir.dt.bfloat16
    f16 = mybir.dt.float16
    f32 = mybir.dt.float32
    i32 = mybir.dt.int32
    DR = mybir.MatmulPerfMode.DoubleRow
    NL = capA + capB

    nwb = len(WB_JBS)
    nc = bacc.Bacc(
        "TRN2",
        target_bir_lowering=False,
        debug=False,
        num_devices=NCORES,
        num_swdge_queues=max(1, nwb),
    )

    # pxt = piS | X in (d, tc, n) layout: one contiguous DMA covers piS and
    # the first-half (d<4) warp operand, so the warp starts one transfer in.
    PXT = 1024 + 16 * NL
    pxt_d = nc.dram_tensor("pxt", [128, PXT], f8, kind="ExternalInput")
    ytl_d = nc.dram_tensor("ytl", [128, 16 * NYL], f8, kind="ExternalInput")
    aux_d = nc.dram_tensor("aux", [4, NYL + NL + 16], f16, kind="ExternalInput")
    out_d = nc.dram_tensor("out", [NYL, NL], bf, kind="ExternalOutput")

    with tile.TileContext(nc) as tc:
        with (
            tc.tile_pool(name="io", bufs=1) as pio,
            tc.tile_pool(name="work", bufs=1) as pw,
            tc.tile_pool(name="small", bufs=1) as psm,
            tc.tile_pool(name="ps", bufs=1, space="PSUM") as pp,
        ):
            pxt = pio.tile([128, PXT], f8, tag="pxt")
            ytl = pio.tile([128, 16 * NYL], f8, tag="ytl")
            aux = psm.tile([4, NYL + NL + 16], f16, tag="aux")
            outsb = pw.tile([128, 4 * NL], bf, tag="outsb")

            piSv = pxt[:, 0:1024].rearrange("l (c t p) -> l c t p", c=2, t=2)
            xt2v = pxt[:, 1024:PXT].rearrange("l (d t n) -> l d t n", d=8, t=2)
            ytlv = ytl.rearrange("l (jb kc j) -> l jb kc j", jb=4, kc=16)

            augL = aux[0:3, 0:NYL]            # [c2A | c2B | ones] over j
            augR = aux[0:3, NYL : NYL + NL]   # [indA | indB | c1c] over n

            # ---- writeback preps (descriptor gen; data read at trigger) ---
            wb_sems, wb_prep, wb_trg = {}, {}, {}
            if nwb:
                idxs = psm.tile([128, 2], i32, tag="wbidx")
                nc.gpsimd.memset(idxs[:], 0)
                outv = outsb.rearrange("j (jb o b n) -> j jb o b n", jb=4, o=1, b=2)
                odv = out_d.rearrange("(jb j o) (b n) -> jb b j o n", jb=4, o=1, b=2)
                for jb in sorted(WB_JBS):
                    qn = sorted(WB_JBS).index(jb)
                    sem = nc.alloc_semaphore(f"wbdma{jb}")
                    wb_sems[jb] = sem
                    wb_prep[jb] = nc.gpsimd.kv_writeback(
                        odv[jb],
                        outv[:, jb],
                        idxs[:],
                        prepare_only=True,
                        sem=sem,
                        queue_num=qn,
                    ).ins

            # ---- input DMAs (all SP HWDGE, wire order = emission order) ---
            HPX = 1024 + 8 * NL  # piS + d<4 half of X
            if getattr(sys.modules[__name__], "PXT_SPLIT3", False):
                HP1 = 1024 + 4 * NL  # piS + d0,d1
                nc.sync.dma_start(pxt[:, 0:HP1], pxt_d[:, 0:HP1])
                nc.sync.dma_start(pxt[:, HP1:HPX], pxt_d[:, HP1:HPX])
            else:
                nc.sync.dma_start(pxt[:, 0:HPX], pxt_d[:, 0:HPX])
            nc.sync.dma_start(pxt[:, HPX:PXT], pxt_d[:, HPX:PXT])
            nc.sync.dma_start(aux[:], aux_d[:, :])
            ytldv = ytl_d.rearrange("l (jb x) -> l jb x", jb=4)
            ytlsv = ytl.rearrange("l (jb x) -> l jb x", jb=4)
            for jb in range(4):
                nc.sync.dma_start(ytlsv[:, jb], ytldv[:, jb])

            # ---- PE p-state priming (dummy matmuls on scratch) ------------
            # pe_busy_start is pinned by the FIRST matmul and survives sub-us
            # idle gaps; a train of cheap dummies bridges until real work so
            # the 3us ramp elapses before the warp starts.
            if N_PRIME:
                dum = psm.tile([128, 256], f8, tag="dum")
                nc.vector.memset(dum[:], 1.0)
                dumv = dum.rearrange("l (t o) -> l t o", o=128)
                # all dummies hit ONE psum tile: the WAW chain is free on the
                # in-order PE, while buffer rotation would add sem waits.
                dps = pp.tile([1, 128], f32, tag="psO", bufs=3, name="dps")
                for i in range(N_PRIME):
                    nc.tensor.matmul(
                        dps[:], dumv[:, :, 0:1], dumv,
                        start=True, stop=True, perf_mode=DR,
                        skip_group_check=True,
                    )

            # ---- aug matmuls: psum group starters -------------------------
            # psO bufs=3 holds jb0-2; outps3 is allocated from the psW pool
            # (by the time its WAR resolves the warp psum cycle is drained).
            outps = [
                pp.tile([128, NL], f32, tag="psO", bufs=3, name=f"outps{jb}")
                for jb in range(3)
            ]

            def aug(jb):
                nc.tensor.matmul(
                    outps[jb][:],
                    augL[:, jb * 128 : (jb + 1) * 128],
                    augR,
                    start=True, stop=False, skip_group_check=True,
                )

            # ---- warp (PE) + contiguous evacs -----------------------------
            # 32 slot-matmuls of 144 cols, grouped 3-per-psum-tile (432 cols)
            # so each evac is one contiguous [128, 432] copy into xwt.
            # Group g covers units [3g, 3g+3), unit u = kc*2 + s.
            # Emission order: groups that only need the first pxt DMA first.
            xwt = pw.tile([128, 16 * NL], f8, tag="xwt")
            units = [(u // 2 // 8, (u // 2) % 8, u % 2) for u in range(32)]
            if getattr(sys.modules[__name__], "G2", False):
                gorder = [0, 1, 6, 7, 2, 3, 8, 9, 4, 5, 10]
            else:
                gorder = [0, 1, 6, 7, 2, 3, 4, 5, 8, 9, 10]

            # warp psum tiles cycle through BOTH pools (the jb psums are idle
            # until the augs, which anyway wait for the warp to drain) -- 8
            # effective slots instead of 5 keeps the WAR loop off the path.
            WTAG = ["psW", "psW", "psW", "psW", "psW", "psO", "psO", "psO",
                    "psW", "psW", "psW"]

            caps = (capA, capB)
            uoff = [0]
            for u in range(32):
                uoff.append(uoff[-1] + caps[u % 2])

            def warp(ei, g):
                u0 = 3 * g
                grp = units[u0 : u0 + 3]
                gw = uoff[u0 + len(grp)] - uoff[u0]
                w = pp.tile(
                    [128, gw], f32, tag=WTAG[ei],
                    bufs=5 if WTAG[ei] == "psW" else 3, name=f"xw{g}"
                )
                c0 = 0
                for i, (pc, d, s) in enumerate(grp):
                    so = s * capA
                    nc.tensor.matmul(
                        w[:, c0 : c0 + caps[s]],
                        piSv[:, s, :, pc * 128 : (pc + 1) * 128],
                        xt2v[:, d, :, so : so + caps[s]],
                        start=True, stop=True, perf_mode=DR,
                        skip_group_check=True,
                    )
                    c0 += caps[s]
                _copy(nc, XW_EVAC[ei],
                      xwt[:, uoff[u0] : uoff[u0] + gw], w[:])

            for ei, g in enumerate(gorder):
                warp(ei, g)
            outps.append(pp.tile([128, NL], f32, tag="psW", bufs=5, name="outps3"))
            for jb in range(4):
                aug(jb)
            xwtv = xwt.rearrange("l (kc n) -> l kc n", kc=16)

            # ---- C3: 8 DR passes per j-block, ordered by when the warp
            # groups covering each kc-pair land (see gorder). Rounds are
            # interleaved across j-blocks so only the last two rounds sit
            # behind the final warp evacs; jb3's early rounds slot in after
            # its (last) ytl chunk arrives.
            if getattr(sys.modules[__name__], "G2", False):
                korder = [0, 5, 1, 2, 6, 3, 4, 7]
            else:
                korder = [0, 5, 1, 2, 3, 4, 6, 7]

            def c3_pass(jb, ki):
                k = korder[ki]
                nc.tensor.matmul(
                    outps[jb][:],
                    ytlv[:, jb, 2 * k : 2 * k + 2, :],
                    xwtv[:, 2 * k : 2 * k + 2, :],
                    start=False, stop=(ki == 7), perf_mode=DR,
                    skip_group_check=True,
                )

            def c3_finish(jb):
                if jb == 3 and getattr(sys.modules[__name__], "JB3_SPLIT", False):
                    evs = [
                        _copy(nc, "a", outsb[:, jb * NL : jb * NL + capA],
                              outps[jb][:, 0:capA]),
                        _copy(nc, "d", outsb[:, jb * NL + capA : (jb + 1) * NL],
                              outps[jb][:, capA:NL]),
                    ]
                else:
                    evs = [_copy(nc, OUT_EVAC[jb],
                                 outsb[:, jb * NL : (jb + 1) * NL], outps[jb][:])]
                if jb in wb_sems:
                    # Drop the bogus WAR edge evac->prep (Tile attributes the
                    # prep's deferred outsb read to DMA completion, which
                    # would deadlock against the evac that PRODUCES the data).
                    # Real ordering: evac -> (sync dep) -> trigger -> DMA read.
                    qn = sorted(WB_JBS).index(jb)
                    dep = _br.InstructionNameOrderedSet()
                    for ev in evs:
                        ev.ins.remove_dependency(wb_prep[jb].name)
                        dep.add(ev.ins.name)
                    trg = nc.gpsimd.trigger_dma(count=None, queue_num=qn)
                    trg.ins.add_sync_dependencies_from(dep)
                    wb_trg[jb] = trg.ins
                else:
                    nc.sync.dma_start(
                        out_d[jb * 128 : (jb + 1) * 128, :],
                        outsb[:, jb * NL : (jb + 1) * NL],
                    )

            order = getattr(sys.modules[__name__], "C3_ORDER", 2)
            if order == 0:
                for ki in range(8):
                    for jb in range(3):
                        c3_pass(jb, ki)
                for jb in range(3):
                    c3_finish(jb)
                for ki in range(8):
                    c3_pass(3, ki)
                c3_finish(3)
            elif order == 1:
                # jb3's early rounds slot between jb0-2's finishes
                for ki in range(8):
                    for jb in range(3):
                        c3_pass(jb, ki)
                for ki in range(6):
                    c3_pass(3, ki)
                for jb in range(3):
                    c3_finish(jb)
                for ki in (6, 7):
                    c3_pass(3, ki)
                c3_finish(3)
            else:
                # jb0 drains fully first so its evac/trigger leave earliest
                for ki in range(8):
                    c3_pass(0, ki)
                c3_finish(0)
                for ki in range(8):
                    for jb in (1, 2):
                        c3_pass(jb, ki)
                for jb in (1, 2):
                    c3_finish(jb)
                for ki in range(8):
                    c3_pass(3, ki)
                c3_finish(3)

            # end-of-kernel: hold Pool until every writeback DMA completed
            # (replaces the DMASW lane waits stripped below, which the
            # timeline scheduler cannot satisfy for user-sem'd preps).
            # All completion waits anchor AFTER the LAST trigger: a wait
            # placed between triggers would serialize them by ~1us each.
            last_jb = max(WB_JBS) if nwb else None
            wge_names = {}
            for jb in sorted(WB_JBS):
                wge = nc.gpsimd.wait_ge(wb_sems[jb], 16)
                wge_names[jb] = wge.ins.name
                dep = _br.InstructionNameOrderedSet()
                dep.add(wb_trg[jb].name)
                dep.add(wb_trg[last_jb].name)
                wge.ins.add_sync_dependencies_from(dep)
            for jb in sorted(WB_JBS):
                desc = _br.InstructionNameOrderedSet()
                desc.add(wge_names[jb])
                if jb == last_jb:
                    for j2 in sorted(WB_JBS):
                        desc.add(wge_names[j2])
                wb_trg[jb].descendants = desc

    if nwb:
        for b in nc.m.functions[0].blocks:
            for i in b.instructions:
                si = i.sync_info
                if si is None:
                    continue
                ws = list(si.on_wait)
                if any("DMASW" in str(w) for w in ws):
                    si.on_wait = [w for w in ws if "DMASW" not in str(w)]

    nc.compile()
    return nc


def kernel(X, Y, pi_dtw, classes):
    import ml_dtypes
    from concourse.bass_utils import run_bass_kernel_spmd

    f8 = ml_dtypes.float8_e4m3
    X = np.ascontiguousarray(np.asarray(X, dtype=np.float32))
    Y = np.ascontiguousarray(np.asarray(Y, dtype=np.float32))
    pi_dtw = np.ascontiguousarray(np.asarray(pi_dtw, dtype=np.float32))
    classes = np.asarray(classes).astype(np.int64)

    counts = np.bincount(classes, minlength=C)
    # slot A holds the larger class of each pair, slot B the smaller, so the
    # SPMD-shared caps (and with them every evac/pass column count) shrink.
    pairs = [(2 * r, 2 * r + 1) for r in range(4)]
    slots = [(a, b) if counts[a] >= counts[b] else (b, a) for a, b in pairs]
    rup = lambda v, m: int(-(-int(v) // m) * m)
    capA = rup(max(counts[a] for a, b in slots), 16)
    capB = rup(max(counts[b] for a, b in slots), 4)
    NL = capA + capB

    if (capA, capB) not in _cache:
        _cache[(capA, capB)] = _build(capA, capB)
    nc = _cache[(capA, capB)]

    idx = [np.nonzero(classes == c)[0] for c in range(C)]

    # bias terms (host): row/col norms contracted with the pi sums
    qfull = (Y * Y).sum(axis=2)          # [NY, TP]
    rfull = (X * X).sum(axis=2)          # [N, T]
    colsum = pi_dtw.sum(axis=1)          # [C, TP]
    rowsum = pi_dtw.sum(axis=2)          # [C, T]
    C2 = qfull @ colsum.T                # [NY, C]
    C1 = (rfull * rowsum[classes]).sum(axis=1)  # [N]

    # per j-half: ytl (-2Y, [p_in, jb, pc, d, jj])
    ytls = []
    for cj in range(2):
        Yh = -2.0 * Y[cj * NYL : (cj + 1) * NYL]
        B = Yh.reshape(4, 128, 2, 128, D).transpose(3, 0, 2, 4, 1)
        ytls.append(np.ascontiguousarray(B.reshape(128, 16 * NYL)).astype(f8))

    in_maps = []
    for r in range(4):
        ca, cb = slots[r]
        Xp = np.zeros((NL, T, D), dtype=np.float32)
        Xp[0 : counts[ca]] = X[idx[ca]]
        Xp[capA : capA + counts[cb]] = X[idx[cb]]
        # [t_in, d, tc, n]
        xt2 = Xp.reshape(NL, 2, 128, D).transpose(2, 3, 1, 0).reshape(128, 16 * NL)

        P = pi_dtw[[ca, cb]]
        pis = P.reshape(2, 2, 128, 256).transpose(2, 0, 1, 3).reshape(128, 1024)
        pxt = np.ascontiguousarray(
            np.concatenate([pis, xt2], axis=1)
        ).astype(f8)

        c1c = np.zeros(NL, dtype=np.float32)
        c1c[0 : counts[ca]] = C1[idx[ca]]
        c1c[capA : capA + counts[cb]] = C1[idx[cb]]

        for cj in range(2):
            aux = np.zeros((4, NYL + NL + 16), dtype=np.float16)
            aux[0, 0:NYL] = C2[cj * NYL : (cj + 1) * NYL, ca]
            aux[1, 0:NYL] = C2[cj * NYL : (cj + 1) * NYL, cb]
            aux[2, 0:NYL] = 1.0
            aux[0, NYL : NYL + counts[ca]] = 1.0  # indA
            aux[1, NYL + capA : NYL + capA + counts[cb]] = 1.0  # indB
            aux[2, NYL : NYL + NL] = c1c
            in_maps.append(
                {"pxt": pxt, "ytl": ytls[cj], "aux": aux}
            )

    res = run_bass_kernel_spmd(nc, in_maps, core_ids=list(range(NCORES)))

    out = np.empty((N, NY), dtype=np.float32)
    jr = [np.arange(0, NYL), np.arange(NYL, NY)]
    for r in range(4):
        ca, cb = slots[r]
        for cj in range(2):
            blk = np.asarray(res.results[2 * r + cj]["out"]).astype(np.float32)
            out[np.ix_(idx[ca], jr[cj])] = blk[:, 0 : counts[ca]].T
            out[np.ix_(idx[cb], jr[cj])] = blk[:, capA : capA + counts[cb]].T
    return out
